# revision 31
# baseline (speedup 1.0000x reference)
"""QRNN (2-layer, forget-mult recurrence) Trainium2 kernel.

Sharding: data-parallel over batch (B=8), one batch element per NeuronCore.
Per-core plan (hardcoded shapes):
  - input x = concat(prev_actions, states, metadata) -> [T=2048, D=1591],
    host-transposed to feature-major [D_pad=1664, T] so the matmul
    contraction dim (features) sits on SBUF partitions and time on the
    free axis.
  - layer l: y = W_l^T @ xT  ([3072, T] in hidden-major layout), computed
    as 128x128 fp32r matmuls accumulating over k-tiles into PSUM [128,512].
  - gates: z = relu(Z + bz), f = sigmoid(F + bf), o = sigmoid(O + bo) on
    ScalarE straight out of PSUM; recurrence c_t = (1-f_t) c_{t-1} + f_t z_t
    via VectorE tensor_tensor_scan along the time (free) axis.
  - layer output o * c is produced in hidden-major layout == exactly the
    feature-major input layout layer 2 needs; final output is transposed
    back on the host.
  - time processed in 2 chunks of 1024 to bound SBUF; scan state carried
    across chunks via a persistent [128,1] initial per hidden tile.
"""

import sys

if "/opt/trn_rl_repo" not in sys.path:
    sys.path.insert(0, "/opt/trn_rl_repo")

import numpy as np

P = 128
T = 2048
TC = 1024          # time chunk
NCHUNK = T // TC
NSUB = TC // 512   # 512-wide matmul moving-dim subtiles per chunk
H = 1024
JT = H // P        # hidden tiles (also layer-2 k-tiles)
K1 = 13            # ceil(1591/128)
D_IN = 1591
D_PAD = K1 * P
K2 = JT
M_TILES = 24       # 3*H / P

TRACE = False
LAST_EXEC_NS = None
LAST_RESULTS = None

_built = None


def _build():
    global _built
    if _built is not None:
        return _built

    import concourse.bass as bass  # noqa: F401
    import concourse.tile as tile
    from concourse import bacc, mybir

    fp32 = mybir.dt.float32
    fp32r = mybir.dt.float32r
    AF = mybir.ActivationFunctionType
    OP = mybir.AluOpType

    nc = bacc.Bacc(None, target_bir_lowering=False, debug=False)

    x_d = nc.declare_dram_parameter("xT", [K1, P, T], fp32r, isOutput=False)
    w1_d = nc.declare_dram_parameter("w1", [M_TILES, P, K1, P], fp32r, isOutput=False)
    b1_d = nc.declare_dram_parameter("b1t", [P, M_TILES], fp32, isOutput=False)
    w2_d = nc.declare_dram_parameter("w2", [M_TILES, P, K2, P], fp32r, isOutput=False)
    b2_d = nc.declare_dram_parameter("b2t", [P, M_TILES], fp32, isOutput=False)
    out_d = nc.declare_dram_parameter("outT", [JT, P, T], fp32, isOutput=True)
    hl_d = nc.declare_dram_parameter("hl", [P, 2, JT], fp32, isOutput=True)

    with tile.TileContext(nc) as tc:
        with (
            tc.tile_pool(name="singles", bufs=1) as singles,
            tc.tile_pool(name="xp", bufs=1) as xp,
            tc.tile_pool(name="x2p", bufs=1) as x2p,
            tc.tile_pool(name="wp", bufs=3) as wp,
            tc.tile_pool(name="temps", bufs=2) as temps,
            tc.tile_pool(name="psum", bufs=8, space="PSUM") as psum,
        ):
            b1_sb = singles.tile([P, M_TILES], fp32)
            b2_sb = singles.tile([P, M_TILES], fp32)
            carry = singles.tile([P, 2, JT], fp32)
            hl_sb = singles.tile([P, 2, JT], fp32)
            nc.sync.dma_start(b1_sb, b1_d.ap())
            nc.sync.dma_start(b2_sb, b2_d.ap())

            w_prefetch = {}
            x_pending = None
            for c in range(NCHUNK):
                tsl = slice(c * TC, (c + 1) * TC)
                if x_pending is not None:
                    x_t = x_pending
                    x_pending = None
                else:
                    x_t = xp.tile([P, K1, TC], fp32r, tag="x")
                if c == 0:
                    # critical-path order: j0/j1 weights first (ACT ring), then
                    # x in k-tile groups (SP ring) so the k-outer matmuls can
                    # start as soon as the first feature tiles land
                    for part in ("z", "f", "o"):
                        mt = {"z": 0, "f": JT, "o": 2 * JT}[part]
                        w_t = wp.tile(
                            [P, K1, P], fp32r, tag="w0", bufs=5, name="w_t"
                        )
                        nc.scalar.dma_start(w_t, w1_d.ap()[mt])
                        w_prefetch[(0, part)] = w_t
                    # deliver x per (half-T, k-group) so the first tile group's
                    # 3-bank k-outer matmuls consume tiles at DMA delivery rate
                    for half in range(NSUB):
                        hsl = slice(c * TC + half * 512, c * TC + (half + 1) * 512)
                        for k0, k1 in ((0, 1), (1, 2), (2, 4), (4, 7), (7, 10), (10, 13)):
                            nc.sync.dma_start(
                                x_t[:, k0:k1, half * 512 : (half + 1) * 512],
                                x_d.ap()[k0:k1, :, hsl].rearrange("ko p t -> p ko t"),
                            )
                x2_t = x2p.tile([P, JT, TC], fp32r, tag="x2")

                for layer in range(2):
                    w_d, b_sb, kt = (
                        (w1_d, b1_sb, K1) if layer == 0 else (w2_d, b2_sb, K2)
                    )
                    for j in range(JT):
                        last_group = (
                            c == NCHUNK - 1 and layer == 1 and j == JT - 1
                        )
                        parts = (("z", 0), ("f", 1), ("o", 2))
                        gates = {}
                        w_ts = {}
                        for part, pidx in parts:
                            mt = pidx * JT + j
                            if c == 0 and layer == 0 and j == 0:
                                w_t = w_prefetch[(j, part)]
                            else:
                                w_t = wp.tile(
                                    [P, kt, P],
                                    fp32r,
                                    tag=f"w{layer}",
                                    bufs=5 if layer == 0 else 6,
                                    name="w_t",
                                )
                                # weight loads ride the ACT HWDGE ring so they
                                # never queue behind the x-chunk stream (SP ring)
                                nc.scalar.dma_start(w_t, w_d.ap()[mt])
                            w_ts[part] = w_t
                            gates[part] = temps.tile([P, TC], fp32, tag=part, name=part)
                        k_outer_first = c == 0 and layer == 0 and j == 0
                        pss = {}
                        for part, pidx in parts:
                            for n in range(NSUB):
                                pss[(part, n)] = psum.tile(
                                    [P, 512], fp32, tag="ps", name="ps"
                                )

                        def emit_mm(part, n, k):
                            rhs_t = x_t if layer == 0 else x2_t
                            nc.tensor.matmul(
                                pss[(part, n)],
                                w_ts[part][:, k, :],
                                rhs_t[:, k, n * 512 : (n + 1) * 512],
                                start=(k == 0),
                                stop=(k == kt - 1),
                            )

                        if k_outer_first:
                            for n in range(NSUB):
                                if n == 0:
                                    # stagger parts by 3 k-tiles so f/o first
                                    # matmuls queue after their weight tiles
                                    # land (in-order PE queue head-of-line)
                                    for s in range(kt + 7):
                                        for pi, (part, _) in enumerate(parts):
                                            k = s - (0, 4, 7)[pi]
                                            if 0 <= k < kt:
                                                emit_mm(part, n, k)
                                else:
                                    for k in range(kt):
                                        for part, _ in parts:
                                            emit_mm(part, n, k)
                        else:
                            for part, _ in parts:
                                for n in range(NSUB):
                                    for k in range(kt):
                                        emit_mm(part, n, k)
                        for part, pidx in parts:
                            mt = pidx * JT + j
                            for n in range(NSUB):
                                nc.scalar.activation(
                                    gates[part][:, n * 512 : (n + 1) * 512],
                                    pss[(part, n)],
                                    AF.Relu if part == "z" else AF.Sigmoid,
                                    bias=b_sb[:, mt : mt + 1],
                                )
                        z, f, o = gates["z"], gates["f"], gates["o"]
                        a_t = temps.tile([P, TC], fp32, tag="a", bufs=1)
                        bb_t = temps.tile([P, TC], fp32, tag="bb", bufs=1)
                        cc_t = temps.tile([P, TC], fp32, tag="cc", bufs=1)
                        # the final tile's epilogue is the kernel tail: split
                        # it per 512-half so the DVE chain + store overlap
                        segs = (
                            [(0, 512), (512, TC)] if last_group else [(0, TC)]
                        )
                        for lo, hi in segs:
                            ss = slice(lo, hi)
                            nc.vector.tensor_scalar(
                                a_t[:, ss], f[:, ss], -1.0, 1.0, OP.mult, OP.add
                            )
                            nc.vector.tensor_mul(bb_t[:, ss], f[:, ss], z[:, ss])
                            if lo == 0:
                                init = 0.0 if c == 0 else carry[:, layer, j : j + 1]
                            else:
                                init = cc_t[:, lo - 1 : lo]
                            nc.vector.tensor_tensor_scan(
                                cc_t[:, ss], a_t[:, ss], bb_t[:, ss], init,
                                OP.mult, OP.add,
                            )
                            if last_group:
                                o2s_t = temps.tile([P, 512], fp32, tag="o2s")
                                nc.vector.tensor_mul(o2s_t, o[:, ss], cc_t[:, ss])
                                # SP ring is idle at the tail; lower trigger+
                                # first-byte latency than SWDGE
                                nc.sync.dma_start(
                                    out_d.ap()[j, :, c * TC + lo : c * TC + hi],
                                    o2s_t,
                                )
                        if c < NCHUNK - 1:
                            nc.vector.tensor_copy(
                                carry[:, layer, j : j + 1], cc_t[:, TC - 1 : TC]
                            )
                        else:
                            nc.vector.tensor_copy(
                                hl_sb[:, layer, j : j + 1], cc_t[:, TC - 1 : TC]
                            )
                        if layer == 0:
                            nc.vector.tensor_mul(x2_t[:, j, :], o, cc_t)
                        elif not last_group:
                            o2_t = temps.tile([P, TC], fp32, tag="o2", bufs=2)
                            nc.vector.tensor_mul(o2_t, o, cc_t)
                            # stores ride SWDGE so they stay off both load rings
                            nc.gpsimd.dma_start(out_d.ap()[j, :, tsl], o2_t)
                        if layer == 1 and j == 0 and c < NCHUNK - 1:
                            # prefetch next chunk's x behind the first output
                            # store on the SWDGE ring: delays its bandwidth
                            # draw past the W2-critical window, still lands
                            # ~50us before layer 1 of the next chunk needs it
                            ntsl = slice((c + 1) * TC, (c + 2) * TC)
                            x_pending = xp.tile([P, K1, TC], fp32r, tag="x")
                            nc.gpsimd.dma_start(
                                x_pending,
                                x_d.ap()[:, :, ntsl].rearrange("ko p t -> p ko t"),
                            )

            nc.sync.dma_start(hl_d.ap(), hl_sb)

    nc.compile()
    _built = nc
    return nc


def _prep_inputs(prev_actions, states, metadata, W1, b1, W2, b2):
    x = np.concatenate(
        [
            np.asarray(prev_actions, np.float32),
            np.asarray(states, np.float32),
            np.asarray(metadata, np.float32),
        ],
        axis=-1,
    )  # [B, T, D_IN]
    B = x.shape[0]
    xT = np.zeros((B, D_PAD, T), np.float32)
    xT[:, :D_IN, :] = x.transpose(0, 2, 1)
    xT = np.ascontiguousarray(xT.reshape(B, K1, P, T))

    W1p = np.zeros((D_PAD, 3 * H), np.float32)
    W1p[:D_IN] = np.asarray(W1, np.float32)
    w1s = np.ascontiguousarray(
        W1p.reshape(K1, P, M_TILES, P).transpose(2, 1, 0, 3)
    )
    w2s = np.ascontiguousarray(
        np.asarray(W2, np.float32).reshape(K2, P, M_TILES, P).transpose(2, 1, 0, 3)
    )
    b1t = np.ascontiguousarray(np.asarray(b1, np.float32).reshape(M_TILES, P).T)
    b2t = np.ascontiguousarray(np.asarray(b2, np.float32).reshape(M_TILES, P).T)
    return xT, w1s, b1t, w2s, b2t


def kernel(prev_actions, states, metadata, W1, b1, W2, b2):
    global LAST_EXEC_NS, LAST_RESULTS
    from concourse.bass_utils import run_bass_kernel_spmd

    xT, w1s, b1t, w2s, b2t = _prep_inputs(
        prev_actions, states, metadata, W1, b1, W2, b2
    )
    B = xT.shape[0]
    nc = _build()
    in_maps = [
        {"xT": xT[i], "w1": w1s, "b1t": b1t, "w2": w2s, "b2t": b2t}
        for i in range(B)
    ]
    res = run_bass_kernel_spmd(nc, in_maps, list(range(B)), trace=TRACE)
    LAST_EXEC_NS = res.exec_time_ns
    LAST_RESULTS = res

    outs = np.empty((B, T, H), np.float32)
    hs = np.empty((B, 2, H), np.float32)
    for i in range(B):
        outT = res.results[i]["outT"].reshape(H, T)
        outs[i] = outT.T
        hs[i] = res.results[i]["hl"].transpose(1, 2, 0).reshape(2, H)
    return outs, hs


# revision 32
# speedup vs baseline: 1.1909x; 1.1909x over previous
"""QRNN (2-layer, forget-mult recurrence) Trainium2 kernel.

Sharding: data-parallel over batch (B=8), one batch element per NeuronCore.
Per-core plan (hardcoded shapes):
  - input x = concat(prev_actions, states, metadata) -> [T=2048, D=1591],
    host-transposed to feature-major [D_pad=1664, T] so the matmul
    contraction dim (features) sits on SBUF partitions and time on the
    free axis.
  - layer l: y = W_l^T @ xT  ([3072, T] in hidden-major layout), computed
    as 128x128 fp32r matmuls accumulating over k-tiles into PSUM [128,512].
  - gates: z = relu(Z + bz), f = sigmoid(F + bf), o = sigmoid(O + bo) on
    ScalarE straight out of PSUM; recurrence c_t = (1-f_t) c_{t-1} + f_t z_t
    via VectorE tensor_tensor_scan along the time (free) axis.
  - layer output o * c is produced in hidden-major layout == exactly the
    feature-major input layout layer 2 needs; final output is transposed
    back on the host.
  - time processed in 2 chunks of 1024 to bound SBUF; scan state carried
    across chunks via a persistent [128,1] initial per hidden tile.
"""

import sys

if "/opt/trn_rl_repo" not in sys.path:
    sys.path.insert(0, "/opt/trn_rl_repo")

import numpy as np

P = 128
T = 2048
TC = 1024          # time chunk
NCHUNK = T // TC
NSUB = TC // 512   # 512-wide matmul moving-dim subtiles per chunk
H = 1024
JT = H // P        # hidden tiles (also layer-2 k-tiles)
K1 = 13            # ceil(1591/128)
D_IN = 1591
D_PAD = K1 * P
K2 = JT
M_TILES = 24       # 3*H / P

TRACE = False
LAST_EXEC_NS = None
LAST_RESULTS = None

_built = None


def _build():
    global _built
    if _built is not None:
        return _built

    import concourse.bass as bass  # noqa: F401
    import concourse.tile as tile
    from concourse import bacc, mybir

    fp32 = mybir.dt.float32
    fp32r = mybir.dt.float32r
    AF = mybir.ActivationFunctionType
    OP = mybir.AluOpType

    nc = bacc.Bacc(None, target_bir_lowering=False, debug=False)

    x_d = nc.declare_dram_parameter("xT", [K1, P, T], fp32r, isOutput=False)
    w1_d = nc.declare_dram_parameter("w1", [M_TILES, P, K1, P], fp32r, isOutput=False)
    b1_d = nc.declare_dram_parameter("b1t", [P, M_TILES], fp32, isOutput=False)
    w2_d = nc.declare_dram_parameter("w2", [M_TILES, P, K2, P], fp32r, isOutput=False)
    b2_d = nc.declare_dram_parameter("b2t", [P, M_TILES], fp32, isOutput=False)
    out_d = nc.declare_dram_parameter("outT", [JT, P, T], fp32, isOutput=True)
    hl_d = nc.declare_dram_parameter("hl", [P, 2, JT], fp32, isOutput=True)

    with tile.TileContext(nc) as tc:
        with (
            tc.tile_pool(name="singles", bufs=1) as singles,
            tc.tile_pool(name="xp", bufs=1) as xp,
            tc.tile_pool(name="x2p", bufs=1) as x2p,
            tc.tile_pool(name="wp", bufs=3) as wp,
            tc.tile_pool(name="temps", bufs=2) as temps,
            tc.tile_pool(name="psum", bufs=8, space="PSUM") as psum,
        ):
            b1_sb = singles.tile([P, M_TILES], fp32)
            b2_sb = singles.tile([P, M_TILES], fp32)
            carry = singles.tile([P, 2, JT], fp32)
            hl_sb = singles.tile([P, 2, JT], fp32)
            nc.sync.dma_start(b1_sb, b1_d.ap())
            nc.sync.dma_start(b2_sb, b2_d.ap())

            w_prefetch = {}
            x_pending = None
            for c in range(NCHUNK):
                tsl = slice(c * TC, (c + 1) * TC)
                if x_pending is not None:
                    x_t = x_pending
                    x_pending = None
                else:
                    x_t = xp.tile([P, K1, TC], fp32r, tag="x")
                if c == 0:
                    # critical-path order: j0/j1 weights first (ACT ring), then
                    # x in k-tile groups (SP ring) so the k-outer matmuls can
                    # start as soon as the first feature tiles land
                    for part in ("z", "f", "o"):
                        mt = {"z": 0, "f": JT, "o": 2 * JT}[part]
                        w_t = wp.tile(
                            [P, K1, P], fp32r, tag="w0", bufs=5, name="w_t"
                        )
                        nc.scalar.dma_start(w_t, w1_d.ap()[mt])
                        w_prefetch[(0, part)] = w_t
                    # deliver x per (half-T, k-group) so the first tile group's
                    # 3-bank k-outer matmuls consume tiles at DMA delivery rate
                    for half in range(NSUB):
                        hsl = slice(c * TC + half * 512, c * TC + (half + 1) * 512)
                        for k0, k1 in ((0, 1), (1, 2), (2, 4), (4, 7), (7, 10), (10, 13)):
                            nc.sync.dma_start(
                                x_t[:, k0:k1, half * 512 : (half + 1) * 512],
                                x_d.ap()[k0:k1, :, hsl].rearrange("ko p t -> p ko t"),
                            )
                x2_t = x2p.tile([P, JT, TC], fp32r, tag="x2")

                for layer in range(2):
                    w_d, b_sb, kt = (
                        (w1_d, b1_sb, K1) if layer == 0 else (w2_d, b2_sb, K2)
                    )
                    for j in range(JT):
                        last_group = (
                            c == NCHUNK - 1 and layer == 1 and j == JT - 1
                        )
                        parts = (("z", 0), ("f", 1), ("o", 2))
                        gates = {}
                        w_ts = {}
                        for part, pidx in parts:
                            mt = pidx * JT + j
                            if c == 0 and layer == 0 and j == 0:
                                w_t = w_prefetch[(j, part)]
                            else:
                                w_t = wp.tile(
                                    [P, kt, P],
                                    fp32r,
                                    tag=f"w{layer}",
                                    bufs=5 if layer == 0 else 6,
                                    name="w_t",
                                )
                                # weight loads ride the ACT HWDGE ring so they
                                # never queue behind the x-chunk stream (SP ring)
                                nc.scalar.dma_start(w_t, w_d.ap()[mt])
                            w_ts[part] = w_t
                            gates[part] = temps.tile([P, TC], fp32, tag=part, name=part)
                        k_outer_first = c == 0 and layer == 0 and j == 0
                        pss = {}
                        for part, pidx in parts:
                            for n in range(NSUB):
                                pss[(part, n)] = psum.tile(
                                    [P, 512], fp32, tag="ps", name="ps"
                                )

                        def emit_mm(part, n, k):
                            rhs_t = x_t if layer == 0 else x2_t
                            nc.tensor.matmul(
                                pss[(part, n)],
                                w_ts[part][:, k, :],
                                rhs_t[:, k, n * 512 : (n + 1) * 512],
                                start=(k == 0),
                                stop=(k == kt - 1),
                            )

                        if k_outer_first:
                            for n in range(NSUB):
                                if n == 0:
                                    # stagger parts by 3 k-tiles so f/o first
                                    # matmuls queue after their weight tiles
                                    # land (in-order PE queue head-of-line)
                                    for s in range(kt + 6):
                                        for pi, (part, _) in enumerate(parts):
                                            k = s - 3 * pi
                                            if 0 <= k < kt:
                                                emit_mm(part, n, k)
                                else:
                                    for k in range(kt):
                                        for part, _ in parts:
                                            emit_mm(part, n, k)
                        else:
                            for part, _ in parts:
                                for n in range(NSUB):
                                    for k in range(kt):
                                        emit_mm(part, n, k)
                        for part, pidx in parts:
                            mt = pidx * JT + j
                            for n in range(NSUB):
                                nc.scalar.activation(
                                    gates[part][:, n * 512 : (n + 1) * 512],
                                    pss[(part, n)],
                                    AF.Relu if part == "z" else AF.Sigmoid,
                                    bias=b_sb[:, mt : mt + 1],
                                )
                        z, f, o = gates["z"], gates["f"], gates["o"]
                        a_t = temps.tile([P, TC], fp32, tag="a", bufs=1)
                        bb_t = temps.tile([P, TC], fp32, tag="bb", bufs=1)
                        cc_t = temps.tile([P, TC], fp32, tag="cc", bufs=1)
                        # the final tile's epilogue is the kernel tail: split
                        # it per 512-half so the DVE chain + store overlap
                        segs = (
                            [(0, 512), (512, TC)] if last_group else [(0, TC)]
                        )
                        for lo, hi in segs:
                            ss = slice(lo, hi)
                            nc.vector.tensor_scalar(
                                a_t[:, ss], f[:, ss], -1.0, 1.0, OP.mult, OP.add
                            )
                            nc.vector.tensor_mul(bb_t[:, ss], f[:, ss], z[:, ss])
                            if lo == 0:
                                init = 0.0 if c == 0 else carry[:, layer, j : j + 1]
                            else:
                                init = cc_t[:, lo - 1 : lo]
                            nc.vector.tensor_tensor_scan(
                                cc_t[:, ss], a_t[:, ss], bb_t[:, ss], init,
                                OP.mult, OP.add,
                            )
                            if last_group:
                                o2s_t = temps.tile([P, 512], fp32, tag="o2s")
                                nc.vector.tensor_mul(o2s_t, o[:, ss], cc_t[:, ss])
                                # SP ring is idle at the tail; lower trigger+
                                # first-byte latency than SWDGE
                                nc.sync.dma_start(
                                    out_d.ap()[j, :, c * TC + lo : c * TC + hi],
                                    o2s_t,
                                )
                        if c < NCHUNK - 1:
                            nc.vector.tensor_copy(
                                carry[:, layer, j : j + 1], cc_t[:, TC - 1 : TC]
                            )
                        else:
                            nc.vector.tensor_copy(
                                hl_sb[:, layer, j : j + 1], cc_t[:, TC - 1 : TC]
                            )
                        if layer == 0:
                            nc.vector.tensor_mul(x2_t[:, j, :], o, cc_t)
                        elif not last_group:
                            o2_t = temps.tile([P, TC], fp32, tag="o2", bufs=2)
                            nc.vector.tensor_mul(o2_t, o, cc_t)
                            # stores ride SWDGE so they stay off both load rings
                            nc.gpsimd.dma_start(out_d.ap()[j, :, tsl], o2_t)
                        if layer == 1 and j == 0 and c < NCHUNK - 1:
                            # prefetch next chunk's x behind the first output
                            # store on the SWDGE ring: delays its bandwidth
                            # draw past the W2-critical window, still lands
                            # ~50us before layer 1 of the next chunk needs it
                            ntsl = slice((c + 1) * TC, (c + 2) * TC)
                            x_pending = xp.tile([P, K1, TC], fp32r, tag="x")
                            nc.gpsimd.dma_start(
                                x_pending,
                                x_d.ap()[:, :, ntsl].rearrange("ko p t -> p ko t"),
                            )

            nc.sync.dma_start(hl_d.ap(), hl_sb)

    nc.compile()
    _built = nc
    return nc


def _prep_inputs(prev_actions, states, metadata, W1, b1, W2, b2):
    x = np.concatenate(
        [
            np.asarray(prev_actions, np.float32),
            np.asarray(states, np.float32),
            np.asarray(metadata, np.float32),
        ],
        axis=-1,
    )  # [B, T, D_IN]
    B = x.shape[0]
    xT = np.zeros((B, D_PAD, T), np.float32)
    xT[:, :D_IN, :] = x.transpose(0, 2, 1)
    xT = np.ascontiguousarray(xT.reshape(B, K1, P, T))

    W1p = np.zeros((D_PAD, 3 * H), np.float32)
    W1p[:D_IN] = np.asarray(W1, np.float32)
    w1s = np.ascontiguousarray(
        W1p.reshape(K1, P, M_TILES, P).transpose(2, 1, 0, 3)
    )
    w2s = np.ascontiguousarray(
        np.asarray(W2, np.float32).reshape(K2, P, M_TILES, P).transpose(2, 1, 0, 3)
    )
    b1t = np.ascontiguousarray(np.asarray(b1, np.float32).reshape(M_TILES, P).T)
    b2t = np.ascontiguousarray(np.asarray(b2, np.float32).reshape(M_TILES, P).T)
    return xT, w1s, b1t, w2s, b2t


def kernel(prev_actions, states, metadata, W1, b1, W2, b2):
    global LAST_EXEC_NS, LAST_RESULTS
    from concourse.bass_utils import run_bass_kernel_spmd

    xT, w1s, b1t, w2s, b2t = _prep_inputs(
        prev_actions, states, metadata, W1, b1, W2, b2
    )
    B = xT.shape[0]
    nc = _build()
    in_maps = [
        {"xT": xT[i], "w1": w1s, "b1t": b1t, "w2": w2s, "b2t": b2t}
        for i in range(B)
    ]
    res = run_bass_kernel_spmd(nc, in_maps, list(range(B)), trace=TRACE)
    LAST_EXEC_NS = res.exec_time_ns
    LAST_RESULTS = res

    outs = np.empty((B, T, H), np.float32)
    hs = np.empty((B, 2, H), np.float32)
    for i in range(B):
        outT = res.results[i]["outT"].reshape(H, T)
        outs[i] = outT.T
        hs[i] = res.results[i]["hl"].transpose(1, 2, 0).reshape(2, H)
    return outs, hs


# revision 33
# speedup vs baseline: 1.1944x; 1.0029x over previous
"""QRNN (2-layer, forget-mult recurrence) Trainium2 kernel.

Sharding: data-parallel over batch (B=8), one batch element per NeuronCore.
Per-core plan (hardcoded shapes):
  - input x = concat(prev_actions, states, metadata) -> [T=2048, D=1591],
    host-transposed to feature-major [D_pad=1664, T] so the matmul
    contraction dim (features) sits on SBUF partitions and time on the
    free axis.
  - layer l: y = W_l^T @ xT  ([3072, T] in hidden-major layout), computed
    as 128x128 fp32r matmuls accumulating over k-tiles into PSUM [128,512].
  - gates: z = relu(Z + bz), f = sigmoid(F + bf), o = sigmoid(O + bo) on
    ScalarE straight out of PSUM; recurrence c_t = (1-f_t) c_{t-1} + f_t z_t
    via VectorE tensor_tensor_scan along the time (free) axis.
  - layer output o * c is produced in hidden-major layout == exactly the
    feature-major input layout layer 2 needs; final output is transposed
    back on the host.
  - time processed in 2 chunks of 1024 to bound SBUF; scan state carried
    across chunks via a persistent [128,1] initial per hidden tile.
"""

import sys

if "/opt/trn_rl_repo" not in sys.path:
    sys.path.insert(0, "/opt/trn_rl_repo")

import numpy as np

P = 128
T = 2048
TC = 1024          # time chunk
NCHUNK = T // TC
NSUB = TC // 512   # 512-wide matmul moving-dim subtiles per chunk
H = 1024
JT = H // P        # hidden tiles (also layer-2 k-tiles)
K1 = 13            # ceil(1591/128)
D_IN = 1591
D_PAD = K1 * P
K2 = JT
M_TILES = 24       # 3*H / P

TRACE = False
LAST_EXEC_NS = None
LAST_RESULTS = None

_built = None


def _build():
    global _built
    if _built is not None:
        return _built

    import concourse.bass as bass  # noqa: F401
    import concourse.tile as tile
    from concourse import bacc, mybir

    fp32 = mybir.dt.float32
    fp32r = mybir.dt.float32r
    AF = mybir.ActivationFunctionType
    OP = mybir.AluOpType

    nc = bacc.Bacc(None, target_bir_lowering=False, debug=False)

    x_d = nc.declare_dram_parameter("xT", [K1, P, T], fp32r, isOutput=False)
    w1_d = nc.declare_dram_parameter("w1", [M_TILES, P, K1, P], fp32r, isOutput=False)
    b1_d = nc.declare_dram_parameter("b1t", [P, M_TILES], fp32, isOutput=False)
    w2_d = nc.declare_dram_parameter("w2", [M_TILES, P, K2, P], fp32r, isOutput=False)
    b2_d = nc.declare_dram_parameter("b2t", [P, M_TILES], fp32, isOutput=False)
    out_d = nc.declare_dram_parameter("outT", [JT, P, T], fp32, isOutput=True)
    hl_d = nc.declare_dram_parameter("hl", [P, 2, JT], fp32, isOutput=True)

    with tile.TileContext(nc) as tc:
        with (
            tc.tile_pool(name="singles", bufs=1) as singles,
            tc.tile_pool(name="xp", bufs=1) as xp,
            tc.tile_pool(name="x2p", bufs=1) as x2p,
            tc.tile_pool(name="wp", bufs=3) as wp,
            tc.tile_pool(name="temps", bufs=2) as temps,
            tc.tile_pool(name="psum", bufs=8, space="PSUM") as psum,
        ):
            b1_sb = singles.tile([P, M_TILES], fp32)
            b2_sb = singles.tile([P, M_TILES], fp32)
            carry = singles.tile([P, 2, JT], fp32)
            hl_sb = singles.tile([P, 2, JT], fp32)
            nc.sync.dma_start(b1_sb, b1_d.ap())
            nc.sync.dma_start(b2_sb, b2_d.ap())

            w_prefetch = {}
            x_pending = None
            for c in range(NCHUNK):
                tsl = slice(c * TC, (c + 1) * TC)
                if x_pending is not None:
                    x_t = x_pending
                    x_pending = None
                else:
                    x_t = xp.tile([P, K1, TC], fp32r, tag="x")
                if c == 0:
                    # critical-path order: j0/j1 weights first (ACT ring), then
                    # x in k-tile groups (SP ring) so the k-outer matmuls can
                    # start as soon as the first feature tiles land
                    for part in ("z", "f", "o"):
                        mt = {"z": 0, "f": JT, "o": 2 * JT}[part]
                        w_t = wp.tile(
                            [P, K1, P], fp32r, tag="w0", bufs=5, name="w_t"
                        )
                        nc.scalar.dma_start(w_t, w1_d.ap()[mt])
                        w_prefetch[(0, part)] = w_t
                    # deliver x per (half-T, k-group) so the first tile group's
                    # 3-bank k-outer matmuls consume tiles at DMA delivery rate
                    for half in range(NSUB):
                        hsl = slice(c * TC + half * 512, c * TC + (half + 1) * 512)
                        for k0, k1 in ((0, 1), (1, 2), (2, 3), (3, 5), (5, 7), (7, 9), (9, 11), (11, 13)):
                            nc.sync.dma_start(
                                x_t[:, k0:k1, half * 512 : (half + 1) * 512],
                                x_d.ap()[k0:k1, :, hsl].rearrange("ko p t -> p ko t"),
                            )
                x2_t = x2p.tile([P, JT, TC], fp32r, tag="x2")

                for layer in range(2):
                    w_d, b_sb, kt = (
                        (w1_d, b1_sb, K1) if layer == 0 else (w2_d, b2_sb, K2)
                    )
                    for j in range(JT):
                        last_group = (
                            c == NCHUNK - 1 and layer == 1 and j == JT - 1
                        )
                        parts = (("z", 0), ("f", 1), ("o", 2))
                        gates = {}
                        w_ts = {}
                        for part, pidx in parts:
                            mt = pidx * JT + j
                            if c == 0 and layer == 0 and j == 0:
                                w_t = w_prefetch[(j, part)]
                            else:
                                w_t = wp.tile(
                                    [P, kt, P],
                                    fp32r,
                                    tag=f"w{layer}",
                                    bufs=5 if layer == 0 else 6,
                                    name="w_t",
                                )
                                # weight loads ride the ACT HWDGE ring so they
                                # never queue behind the x-chunk stream (SP ring)
                                nc.scalar.dma_start(w_t, w_d.ap()[mt])
                            w_ts[part] = w_t
                            gates[part] = temps.tile([P, TC], fp32, tag=part, name=part)
                        k_outer_first = c == 0 and layer == 0 and j == 0
                        pss = {}
                        for part, pidx in parts:
                            for n in range(NSUB):
                                pss[(part, n)] = psum.tile(
                                    [P, 512], fp32, tag="ps", name="ps"
                                )

                        def emit_mm(part, n, k):
                            rhs_t = x_t if layer == 0 else x2_t
                            nc.tensor.matmul(
                                pss[(part, n)],
                                w_ts[part][:, k, :],
                                rhs_t[:, k, n * 512 : (n + 1) * 512],
                                start=(k == 0),
                                stop=(k == kt - 1),
                            )

                        if k_outer_first:
                            for n in range(NSUB):
                                if n == 0:
                                    # stagger parts by 3 k-tiles so f/o first
                                    # matmuls queue after their weight tiles
                                    # land (in-order PE queue head-of-line)
                                    for s in range(kt + 6):
                                        for pi, (part, _) in enumerate(parts):
                                            k = s - 3 * pi
                                            if 0 <= k < kt:
                                                emit_mm(part, n, k)
                                else:
                                    for k in range(kt):
                                        for part, _ in parts:
                                            emit_mm(part, n, k)
                        else:
                            for part, _ in parts:
                                for n in range(NSUB):
                                    for k in range(kt):
                                        emit_mm(part, n, k)
                        for part, pidx in parts:
                            mt = pidx * JT + j
                            for n in range(NSUB):
                                nc.scalar.activation(
                                    gates[part][:, n * 512 : (n + 1) * 512],
                                    pss[(part, n)],
                                    AF.Relu if part == "z" else AF.Sigmoid,
                                    bias=b_sb[:, mt : mt + 1],
                                )
                        z, f, o = gates["z"], gates["f"], gates["o"]
                        a_t = temps.tile([P, TC], fp32, tag="a", bufs=1)
                        bb_t = temps.tile([P, TC], fp32, tag="bb", bufs=1)
                        cc_t = temps.tile([P, TC], fp32, tag="cc", bufs=1)
                        # the final tile's epilogue is the kernel tail: split
                        # it per 512-half so the DVE chain + store overlap
                        segs = (
                            [(0, 512), (512, TC)] if last_group else [(0, TC)]
                        )
                        for lo, hi in segs:
                            ss = slice(lo, hi)
                            nc.vector.tensor_scalar(
                                a_t[:, ss], f[:, ss], -1.0, 1.0, OP.mult, OP.add
                            )
                            nc.vector.tensor_mul(bb_t[:, ss], f[:, ss], z[:, ss])
                            if lo == 0:
                                init = 0.0 if c == 0 else carry[:, layer, j : j + 1]
                            else:
                                init = cc_t[:, lo - 1 : lo]
                            nc.vector.tensor_tensor_scan(
                                cc_t[:, ss], a_t[:, ss], bb_t[:, ss], init,
                                OP.mult, OP.add,
                            )
                            if last_group:
                                o2s_t = temps.tile([P, 512], fp32, tag="o2s")
                                nc.vector.tensor_mul(o2s_t, o[:, ss], cc_t[:, ss])
                                # SP ring is idle at the tail; lower trigger+
                                # first-byte latency than SWDGE
                                nc.sync.dma_start(
                                    out_d.ap()[j, :, c * TC + lo : c * TC + hi],
                                    o2s_t,
                                )
                        if c < NCHUNK - 1:
                            nc.vector.tensor_copy(
                                carry[:, layer, j : j + 1], cc_t[:, TC - 1 : TC]
                            )
                        else:
                            nc.vector.tensor_copy(
                                hl_sb[:, layer, j : j + 1], cc_t[:, TC - 1 : TC]
                            )
                        if layer == 0:
                            nc.vector.tensor_mul(x2_t[:, j, :], o, cc_t)
                        elif not last_group:
                            o2_t = temps.tile([P, TC], fp32, tag="o2", bufs=2)
                            nc.vector.tensor_mul(o2_t, o, cc_t)
                            # stores ride SWDGE so they stay off both load rings
                            nc.gpsimd.dma_start(out_d.ap()[j, :, tsl], o2_t)
                        if layer == 1 and j == 0 and c < NCHUNK - 1:
                            # prefetch next chunk's x behind the first output
                            # store on the SWDGE ring: delays its bandwidth
                            # draw past the W2-critical window, still lands
                            # ~50us before layer 1 of the next chunk needs it
                            ntsl = slice((c + 1) * TC, (c + 2) * TC)
                            x_pending = xp.tile([P, K1, TC], fp32r, tag="x")
                            nc.gpsimd.dma_start(
                                x_pending,
                                x_d.ap()[:, :, ntsl].rearrange("ko p t -> p ko t"),
                            )

            nc.sync.dma_start(hl_d.ap(), hl_sb)

    nc.compile()
    _built = nc
    return nc


def _prep_inputs(prev_actions, states, metadata, W1, b1, W2, b2):
    x = np.concatenate(
        [
            np.asarray(prev_actions, np.float32),
            np.asarray(states, np.float32),
            np.asarray(metadata, np.float32),
        ],
        axis=-1,
    )  # [B, T, D_IN]
    B = x.shape[0]
    xT = np.zeros((B, D_PAD, T), np.float32)
    xT[:, :D_IN, :] = x.transpose(0, 2, 1)
    xT = np.ascontiguousarray(xT.reshape(B, K1, P, T))

    W1p = np.zeros((D_PAD, 3 * H), np.float32)
    W1p[:D_IN] = np.asarray(W1, np.float32)
    w1s = np.ascontiguousarray(
        W1p.reshape(K1, P, M_TILES, P).transpose(2, 1, 0, 3)
    )
    w2s = np.ascontiguousarray(
        np.asarray(W2, np.float32).reshape(K2, P, M_TILES, P).transpose(2, 1, 0, 3)
    )
    b1t = np.ascontiguousarray(np.asarray(b1, np.float32).reshape(M_TILES, P).T)
    b2t = np.ascontiguousarray(np.asarray(b2, np.float32).reshape(M_TILES, P).T)
    return xT, w1s, b1t, w2s, b2t


def kernel(prev_actions, states, metadata, W1, b1, W2, b2):
    global LAST_EXEC_NS, LAST_RESULTS
    from concourse.bass_utils import run_bass_kernel_spmd

    xT, w1s, b1t, w2s, b2t = _prep_inputs(
        prev_actions, states, metadata, W1, b1, W2, b2
    )
    B = xT.shape[0]
    nc = _build()
    in_maps = [
        {"xT": xT[i], "w1": w1s, "b1t": b1t, "w2": w2s, "b2t": b2t}
        for i in range(B)
    ]
    res = run_bass_kernel_spmd(nc, in_maps, list(range(B)), trace=TRACE)
    LAST_EXEC_NS = res.exec_time_ns
    LAST_RESULTS = res

    outs = np.empty((B, T, H), np.float32)
    hs = np.empty((B, 2, H), np.float32)
    for i in range(B):
        outT = res.results[i]["outT"].reshape(H, T)
        outs[i] = outT.T
        hs[i] = res.results[i]["hl"].transpose(1, 2, 0).reshape(2, H)
    return outs, hs
